# revision 16
# baseline (speedup 1.0000x reference)
"""Trainium2 Bass kernel v3 for nn_ExtensibleAttention.

Math (reference.py):
  q = query@Wq.T + pos@Wp.T ; k = key@Wk.T + pos@Wp.T ; v = value@Wv.T
  sp = reference_points (offsets are zero for this problem)
  k_s, v_s = bilinear_sample(k_map, sp), bilinear_sample(v_map, sp)
  a = (q.k_s)/sqrt(D) per head ; w = softmax over the 8 heads
  out = (w * v_s) @ Wout.T + bout

v3 structure (vs v2 baseline):
  - REGION SHARDING: 8 cores = 4 batches x 2 map-row regions. Core (n,h)
    builds only map rows [h*4608, h*4608+4736) (37 tiles instead of 72)
    and serves the tokens of batch n whose sample row falls in its
    region. Halves the map projection matmuls and the k/v/pos loads.
  - combine rewritten as 6 TS/TT ops (y-pair then x-pair) exploiting the
    DVE 4x TENSOR_SCALAR mode; weights passed as separate sy0/sy1/sx0/
    sx1 vectors instead of 4 products.
  - q projection just-in-time per pair; the q.k mult reads q straight
    from PSUM (no q_all copy).
  - softmax 1/sum folded into the output-projection PSUM->SBUF copy
    (ACT scale), so the weighted-v op is a plain TT with broadcast exp.
  - per-pair batching of tail ops (tmp/reduce/recip/outs) to amortize
    fixed instruction overheads; map copies batched 2 tiles per call.
  - all HWDGE dma_start on the sync queue (ACT queue freed).

Sharding: 8 cores = 4 batches x 2 map regions; tokens sorted by sampled
map row; output unsharded host-side by scatter.
"""

import sys

import numpy as np

if "/opt/trn_rl_repo" not in sys.path:
    sys.path.insert(0, "/opt/trn_rl_repo")

import concourse.bacc as bacc
import concourse.mybir as mybir
import concourse.tile as tile
from concourse import library_config
from concourse.bass_utils import run_bass_kernel_spmd
from concourse.mybir import ActivationFunctionType as AFT
from concourse.mybir import AluOpType as ALU
import bass_rust

F32 = mybir.dt.float32
BF16 = mybir.dt.bfloat16
I16 = mybir.dt.int16
NP_BF16 = mybir.dt.np(BF16)

P = 128
C = 256
CH = 2
H = 8
D = 32
HF = WF = 96
LMAP = HF * WF          # 9216
REG = LMAP // 2         # 4608 rows per region
NT_MAP = 37             # map tiles per core (region + 97-row overlap)
MAP_ROWS = NT_MAP * P   # 4736
ROW = 2 * C             # 512 bf16 elems per map row (k || v)
ELEM = 2 * ROW          # gather element: x-pair, 2 rows
N_CORES = 8
N_BATCH = 4
INV_SQRT_D = 1.0 / np.sqrt(np.float32(D))
MAP_CHUNKS = [4, 5, 9, 9, 10]   # strip 0 split so the first block flushes early


def _q_chunks(nt_q):
    base = nt_q // 4
    rem = nt_q - 4 * base
    return [base + (1 if i < rem else 0) for i in range(4)]


def build_program(nt_q, row_bounds):
    npair = nt_q // 2
    q_chunks = _q_chunks(nt_q)

    nc = bacc.Bacc("TRN2", target_bir_lowering=False, debug=False,
                   num_devices=N_CORES, num_swdge_queues=2)

    keyT = nc.dram_tensor("keyT", [C, MAP_ROWS], BF16, kind="ExternalInput")
    valueT = nc.dram_tensor("valueT", [C, MAP_ROWS], BF16, kind="ExternalInput")
    posT = nc.dram_tensor("posT", [C, MAP_ROWS], BF16, kind="ExternalInput")
    queryT = nc.dram_tensor("queryT", [C, nt_q * P], BF16, kind="ExternalInput")
    posqT = nc.dram_tensor("posqT", [C, nt_q * P], BF16, kind="ExternalInput")
    idxT = nc.dram_tensor("idxT", [npair, P, 32], I16, kind="ExternalInput")
    syxT = nc.dram_tensor("syxT", [nt_q, P, 4], F32, kind="ExternalInput")
    wkT = nc.dram_tensor("wkT", [C, C], BF16, kind="ExternalInput")
    wvT = nc.dram_tensor("wvT", [C, C], BF16, kind="ExternalInput")
    wpT = nc.dram_tensor("wpT", [C, C], BF16, kind="ExternalInput")
    wqT = nc.dram_tensor("wqT", [C, C], BF16, kind="ExternalInput")
    woT = nc.dram_tensor("woT", [C, C], BF16, kind="ExternalInput")
    identity = nc.dram_tensor("identity", [P, P], BF16, kind="ExternalInput")
    out = nc.dram_tensor("out", [nt_q * P, C], BF16, kind="ExternalOutput")

    kv_map = nc.dram_tensor("kv_map", [MAP_ROWS, ROW], BF16, kind="Internal")

    with tile.TileContext(nc) as tc:
        with (
            tc.tile_pool(name="const", bufs=1) as const,
            tc.tile_pool(name="kstrip", bufs=2) as kstrip_p,
            tc.tile_pool(name="vstrip", bufs=2) as vstrip_p,
            tc.tile_pool(name="pstrip", bufs=2) as pstrip_p,
            tc.tile_pool(name="qstrip", bufs=3) as qstrip_p,
            tc.tile_pool(name="pqstrip", bufs=3) as pqstrip_p,
            tc.tile_pool(name="kv", bufs=3) as kv_p,
            tc.tile_pool(name="gat", bufs=6) as gat_p,
            tc.tile_pool(name="att", bufs=3) as att_p,
            tc.tile_pool(name="kvs", bufs=3) as kvs_p,
            tc.tile_pool(name="small", bufs=3) as small_p,
            tc.tile_pool(name="rin", bufs=4) as rin_p,
            tc.tile_pool(name="outs", bufs=4) as outs_p,
            tc.tile_pool(name="obuf", bufs=2) as obuf_p,
            tc.tile_pool(name="psM", bufs=2, space="PSUM") as psM,
            tc.tile_pool(name="psQ", bufs=2, space="PSUM") as psQ,
            tc.tile_pool(name="psT", bufs=1, space="PSUM") as psT,
            tc.tile_pool(name="psF", bufs=1, space="PSUM") as psF,
        ):
            # ---- constants needed by the map/q strips (loaded first) ----
            def load_w(t):
                sb = const.tile([P, CH, C], BF16, tag=f"w_{t.name}")
                nc.scalar.dma_start(sb[:], t.ap().rearrange("(ch p) n -> p ch n", p=P))
                return sb
            wk_sb, wv_sb, wp_sb, wq_sb = (
                load_w(t) for t in (wkT, wvT, wpT, wqT))

            nc.gpsimd.load_library(library_config.mlp)

            # constants only needed once pairs start draining; loaded after
            # the first strips so they don't delay the first map matmuls
            late = {}

            def load_late():
                late["wo_sb"] = load_w(woT)
                ident_sb = const.tile([P, P], BF16, tag="ident")
                nc.scalar.dma_start(ident_sb[:], identity.ap())
                late["ident_sb"] = ident_sb
                idx_sb = const.tile([P, npair, 32], I16, tag="idx")
                nc.scalar.dma_start(idx_sb[:], idxT.ap().rearrange("j p s -> p j s"))
                late["idx_sb"] = idx_sb
                syx = const.tile([P, nt_q, 4], F32, tag="syx")
                nc.scalar.dma_start(syx[:], syxT.ap().rearrange("j p c -> p j c"))
                late["syx"] = syx

            # ---- map strips: project k/v/pos into kv_map rows ----
            def map_strip_loads(s):
                ls = MAP_CHUNKS[s] * P
                t0 = sum(MAP_CHUNKS[:s])
                k_st = kstrip_p.tile([P, CH, 10 * P], BF16, name="k_st")
                v_st = vstrip_p.tile([P, CH, 10 * P], BF16, name="v_st")
                p_st = pstrip_p.tile([P, CH, 10 * P], BF16, name="p_st")
                for st, t in ((k_st, keyT), (p_st, posT), (v_st, valueT)):
                    # input loads ride the scalar HWDGE queue: the sync queue
                    # carries map/out writes and was saturating during the ramp
                    nc.scalar.dma_start(
                        st[:, :, 0:ls],
                        t.ap().rearrange("(ch p) l -> p ch l", p=P)
                        [:, :, t0 * P:t0 * P + ls])
                return k_st, p_st, v_st

            def map_block(strips, s, b):
                k_st, p_st, v_st = strips
                t0 = sum(MAP_CHUNKS[:s])
                cnt = min(2, MAP_CHUNKS[s] - b)
                kv2 = psM.tile([P, 2, ROW], F32, space="PSUM", name="kv2")
                for jj in range(cnt):
                    sl = slice((b + jj) * P, (b + jj + 1) * P)
                    nc.tensor.matmul(kv2[:, jj, 0:C], k_st[:, 0, sl],
                                     wk_sb[:, 0, :], start=True, stop=False)
                    nc.tensor.matmul(kv2[:, jj, 0:C], k_st[:, 1, sl],
                                     wk_sb[:, 1, :], start=False, stop=False)
                    nc.tensor.matmul(kv2[:, jj, 0:C], p_st[:, 0, sl],
                                     wp_sb[:, 0, :], start=False, stop=False)
                    nc.tensor.matmul(kv2[:, jj, 0:C], p_st[:, 1, sl],
                                     wp_sb[:, 1, :], start=False, stop=True)
                    nc.tensor.matmul(kv2[:, jj, C:ROW], v_st[:, 0, sl],
                                     wv_sb[:, 0, :], start=True, stop=False)
                    nc.tensor.matmul(kv2[:, jj, C:ROW], v_st[:, 1, sl],
                                     wv_sb[:, 1, :], start=False, stop=True)
                kvt = kv_p.tile([P, 2, ROW], BF16, name="kvt")
                # map PSUM->SBUF copy on ACT (DVE carries the combine)
                nc.scalar.activation(kvt[:, 0:cnt, :], kv2[:, 0:cnt, :],
                                     AFT.Copy)
                m0 = t0 + b
                nc.sync.dma_start(
                    kv_map.ap()[m0 * P:(m0 + cnt) * P, :]
                    .rearrange("(jj p) e -> p jj e", p=P),
                    kvt[:, 0:cnt, :])
                return cnt

            # ---- q strips: load raw query/pos chunks (projection is JIT) ----
            q_tiles = {}

            def q_strip(s):
                ls = q_chunks[s] * P
                if ls == 0:
                    return
                t0 = sum(q_chunks[:s])
                q_st = qstrip_p.tile([P, CH, 10 * P], BF16, name="q_st")
                pq_st = pqstrip_p.tile([P, CH, 10 * P], BF16, name="pq_st")
                for st, t in ((q_st, queryT), (pq_st, posqT)):
                    nc.scalar.dma_start(
                        st[:, :, 0:ls],
                        t.ap().rearrange("(ch p) l -> p ch l", p=P)
                        [:, :, t0 * P:t0 * P + ls])
                for j in range(t0, t0 + q_chunks[s]):
                    q_tiles[j] = (q_st, pq_st, (j - t0) * P)

            # ---- attention pair: gather + q proj + combine + softmax ----
            pair_state = {}

            def emit_pair(pj):
                idx_sb, syx = late["idx_sb"], late["syx"]
                j0 = 2 * pj
                g = gat_p.tile([P, 4, ELEM], BF16, name="g")
                nrows = int(row_bounds[pj])
                src = bass_rust.AP(tensor=kv_map.ap().tensor, offset=0,
                                   ap=[[ROW, nrows], [1, ELEM]])
                nc.gpsimd.dma_gather(
                    out_ap=g[:],
                    in_ap=src,
                    idxs_ap=idx_sb[:, pj, :],
                    num_idxs=4 * P,
                    num_idxs_reg=4 * P,
                    elem_size=ELEM,
                    elem_step=ROW,
                    queue_num=pj % 2,
                )
                # q projection JIT for both tiles of the pair
                qp = psQ.tile([P, 2, C], F32, space="PSUM", name="qp")
                for jj in range(2):
                    q_st, pq_st, off = q_tiles[j0 + jj]
                    sl = slice(off, off + P)
                    nc.tensor.matmul(qp[:, jj, :], q_st[:, 0, sl], wq_sb[:, 0, :],
                                     start=True, stop=False)
                    nc.tensor.matmul(qp[:, jj, :], q_st[:, 1, sl], wq_sb[:, 1, :],
                                     start=False, stop=False)
                    nc.tensor.matmul(qp[:, jj, :], pq_st[:, 0, sl], wp_sb[:, 0, :],
                                     start=False, stop=False)
                    nc.tensor.matmul(qp[:, jj, :], pq_st[:, 1, sl], wp_sb[:, 1, :],
                                     start=False, stop=True)

                kvs = kvs_p.tile([P, 2, ROW], BF16, name="kvs")
                for jj in range(2):
                    j = j0 + jj
                    g0, g1 = 2 * jj, 2 * jj + 1
                    # 4-corner combine: per-corner TS (DVE 4x mode on [512]),
                    # one corner on ACT to balance; tree of 2x TT adds.
                    # slots: g[g0]=y0 elem (x0|x1 rows), g[g1]=y1 elem.
                    # NB: gpsimd elementwise here forces Pool MODIFY_POOL_CONFIG
                    # swaps against the gather preps and serializes the whole
                    # pipeline (measured 2x slowdown) — keep off gpsimd.
                    cA = att_p.tile([P, ROW], BF16, name="cA", tag="cA")
                    nc.vector.tensor_scalar(out=cA[:], in0=g[:, g0, 0:ROW],
                                            scalar1=syx[:, j, 0:1], scalar2=None,
                                            op0=ALU.mult)
                    cB = att_p.tile([P, ROW], BF16, name="cB", tag="cB")
                    nc.scalar.activation(cB[:], g[:, g0, ROW:ELEM], AFT.Copy,
                                         scale=syx[:, j, 1:2])
                    cC = att_p.tile([P, ROW], BF16, name="cC", tag="cC")
                    nc.vector.tensor_scalar(out=cC[:], in0=g[:, g1, 0:ROW],
                                            scalar1=syx[:, j, 2:3], scalar2=None,
                                            op0=ALU.mult)
                    cD = att_p.tile([P, ROW], BF16, name="cD", tag="cD")
                    nc.scalar.activation(cD[:], g[:, g1, ROW:ELEM], AFT.Copy,
                                         scale=syx[:, j, 3:4])
                    s0 = att_p.tile([P, ROW], BF16, name="s0", tag="s0")
                    nc.vector.tensor_tensor(out=s0[:], in0=cA[:], in1=cC[:],
                                            op=ALU.add)
                    s1 = att_p.tile([P, ROW], BF16, name="s1", tag="s1")
                    nc.vector.tensor_tensor(out=s1[:], in0=cB[:], in1=cD[:],
                                            op=ALU.add)
                    nc.vector.tensor_tensor(out=kvs[:, jj, :], in0=s0[:],
                                            in1=s1[:], op=ALU.add)

                # pair-batched tail
                tmp = att_p.tile([P, 2, C], BF16, name="tmp", tag="tmp")
                nc.vector.tensor_tensor(out=tmp[:], in0=qp[:],
                                        in1=kvs[:, :, 0:C], op=ALU.mult)
                a = small_p.tile([P, 2, H], F32, name="a", tag="a")
                nc.vector.reduce_sum(
                    out=a[:], in_=tmp[:].rearrange("p t (h d) -> p t h d", d=D),
                    axis=mybir.AxisListType.X)
                e = small_p.tile([P, 2, H], F32, name="e", tag="e")
                ssum = small_p.tile([P, 2], F32, name="ssum", tag="ssum")
                for jj in range(2):
                    nc.scalar.activation(e[:, jj, :], a[:, jj, :], AFT.Exp,
                                         scale=float(INV_SQRT_D),
                                         accum_out=ssum[:, jj:jj + 1])
                rinv = rin_p.tile([P, 2], F32, name="rinv")
                nc.vector.reciprocal(rinv[:], ssum[:])
                outs = outs_p.tile([P, 2, C], BF16, name="outs")
                nc.vector.tensor_tensor(
                    out=outs[:].rearrange("p t (h d) -> p t h d", d=D),
                    in0=kvs[:, :, C:ROW].rearrange("p t (h d) -> p t h d", d=D),
                    in1=e[:].to_broadcast([P, 2, H, D]),
                    op=ALU.mult)
                pair_state[pj] = (outs, rinv)

            # ---- output projection per pair (deferred by a small lag) ----
            def emit_att_b(pj):
                ident_sb, wo_sb = late["ident_sb"], late["wo_sb"]
                outs, rinv = pair_state.pop(pj)
                tp2 = psT.tile([P, 2, C], BF16, space="PSUM", name="tp2")
                for jj in range(2):
                    nc.tensor.transpose(tp2[:, jj, 0:P], outs[:, jj, 0:P],
                                        ident_sb[:])
                    nc.tensor.transpose(tp2[:, jj, P:C], outs[:, jj, P:C],
                                        ident_sb[:])
                oT2 = att_p.tile([P, 2, C], BF16, name="oT2", tag="oT2")
                nc.scalar.activation(oT2[:], tp2[:], AFT.Copy)
                fp2 = psF.tile([P, 2, C], F32, space="PSUM", name="fp2")
                for jj in range(2):
                    nc.tensor.matmul(fp2[:, jj, :], oT2[:, jj, 0:P],
                                     wo_sb[:, 0, :], start=True, stop=False)
                    nc.tensor.matmul(fp2[:, jj, :], oT2[:, jj, P:C],
                                     wo_sb[:, 1, :], start=False, stop=True)
                ot2 = obuf_p.tile([P, 2, C], BF16, name="ot2")
                for jj in range(2):
                    nc.scalar.activation(ot2[:, jj, :], fp2[:, jj, :], AFT.Copy,
                                         scale=rinv[:, jj:jj + 1])
                j0 = 2 * pj
                nc.sync.dma_start(
                    out.ap()[j0 * P:(j0 + 2) * P, :]
                    .rearrange("(jj p) e -> p jj e", p=P),
                    ot2[:])

            # ---- schedule: fine-grained map-block / pair interleave ----
            pj = 0
            bj = 0
            flushed = 0

            def drain(limit_rows, q_loaded):
                nonlocal pj, bj
                while (pj < npair and int(row_bounds[pj]) <= limit_rows
                       and 2 * (pj + 1) * P <= q_loaded):
                    emit_pair(pj)
                    pj += 1
                    if pj - 2 > bj:
                        emit_att_b(bj)
                        bj += 1

            q_loaded = 0
            for s in range(len(MAP_CHUNKS)):
                strips = map_strip_loads(s)
                if s < 4:
                    q_strip(s)
                    q_loaded = P * sum(q_chunks[:s + 1])
                if s == 0:
                    load_late()
                for b in range(0, MAP_CHUNKS[s], 2):
                    prev = flushed
                    flushed += P * map_block(strips, s, b)
                    # one-block lag so ACT-queue combine ops don't head-block
                    # behind a still-in-flight gather
                    drain(prev, q_loaded)
            drain(MAP_ROWS, nt_q * P)
            while bj < npair:
                emit_att_b(bj)
                bj += 1

    nc.compile()
    return nc


_PROGRAM = None
_PROGRAM_KEY = None


def _get_program(nt_q, row_bounds):
    global _PROGRAM, _PROGRAM_KEY
    key = (nt_q, tuple(int(b) for b in row_bounds))
    if _PROGRAM is None or _PROGRAM_KEY != key:
        _PROGRAM_KEY = key
        _PROGRAM = build_program(nt_q, row_bounds)
    return _PROGRAM


def host_prep(ref_pts, h):
    """Region prep for one core: tokens of the batch whose bilinear base row
    falls in region h. ref_pts: [9216, 2]. Returns (tok_sorted, count,
    idx_vals [npair-var, 512] builder inputs, syx [count,4], bounds...)"""
    x = ref_pts[:, 0] * np.float32(WF) - np.float32(0.5)
    y = ref_pts[:, 1] * np.float32(HF) - np.float32(0.5)
    x0 = np.floor(x)
    y0 = np.floor(y)
    wx = (x - x0).astype(np.float32)
    wy = (y - y0).astype(np.float32)
    xb = np.clip(x0, 0, WF - 1).astype(np.int32)
    yb = np.clip(y0, 0, HF - 1).astype(np.int32)
    sy0 = np.where(y0 < 0, wy, 1.0 - wy).astype(np.float32)
    sy1 = np.where((y0 < 0) | (y0 >= HF - 1), 0.0, wy).astype(np.float32)
    sx0 = np.where(x0 < 0, wx, 1.0 - wx).astype(np.float32)
    sx1 = np.where((x0 < 0) | (x0 >= WF - 1), 0.0, wx).astype(np.float32)
    r0 = (yb * WF + xb).astype(np.int32)

    sel = np.where((r0 >= h * REG) & (r0 < (h + 1) * REG))[0]
    r0l = r0[sel] - h * REG
    order = np.argsort(r0l, kind="stable")
    tok_sorted = sel[order]
    r0s = r0l[order]
    # corner weights in gather-slot order [y0x0, y0x1, y1x0, y1x1]
    syx4 = np.stack([sy0[tok_sorted] * sx0[tok_sorted],
                     sy0[tok_sorted] * sx1[tok_sorted],
                     sy1[tok_sorted] * sx0[tok_sorted],
                     sy1[tok_sorted] * sx1[tok_sorted]], axis=1)
    return tok_sorted, r0s, syx4


def _build_core_arrays(r0s, syx4, nt_q):
    npair = nt_q // 2
    ntok = nt_q * P
    count = r0s.shape[0]
    r0p = np.zeros(ntok, np.int32)
    r0p[:count] = r0s
    syxp = np.zeros((ntok, 4), np.float32)
    syxp[:count] = syx4
    idx_wrapped = np.zeros((npair, P, 32), np.int16)
    row_bounds = np.zeros((npair,), np.int32)
    for pj in range(npair):
        vals = []
        for half in range(2):
            rt = r0p[(2 * pj + half) * P:(2 * pj + half + 1) * P]
            vals.append(rt)
            vals.append(rt + WF)
        vals = np.concatenate(vals).astype(np.int16)
        buf = np.zeros((P, 32), np.int16)
        ii = np.arange(4 * P)
        buf[ii % 16, ii // 16] = vals
        for k in range(1, 8):
            buf[16 * k:16 * (k + 1)] = buf[0:16]
        idx_wrapped[pj] = buf
        row_bounds[pj] = min(int(vals.max()) + 2, MAP_ROWS - 1)
    return idx_wrapped, syxp.reshape(nt_q, P, 4), row_bounds


def _reference_numpy(query, key, value, reference_points, pos_embed,
                     Wq, bq, Wk, bk, Wv, bv, Wp, bp, Woff, boff, Wout, bout,
                     h_feat, w_feat):
    """Exact numpy fallback (only used for non-matching setups)."""
    N, L, Cc = query.shape
    Hn = H
    Dn = Cc // Hn
    q = (query @ Wq.T + bq).reshape(N, L, Hn, Dn)
    k = (key @ Wk.T + bk).reshape(N, L, Hn, Dn)
    v = (value @ Wv.T + bv).reshape(N, L, Hn, Dn)
    pos = (pos_embed @ Wp.T + bp).reshape(N, L, Hn, Dn)
    q = q + pos
    k = k + pos
    offsets = (query @ Woff.T + boff).reshape(N, L, Hn, 2)
    sp = reference_points[:, :, None, :] + offsets
    k_map = k.reshape(N, h_feat, w_feat, Hn, Dn)
    v_map = v.reshape(N, h_feat, w_feat, Hn, Dn)

    def bil(feat, pts):
        x = pts[..., 0] * w_feat - 0.5
        y = pts[..., 1] * h_feat - 0.5
        x0 = np.floor(x).astype(np.int64)
        y0 = np.floor(y).astype(np.int64)
        wx = x - x0
        wy = y - y0
        res = 0.0
        for yi, xi, wgt in ((y0, x0, (1 - wy) * (1 - wx)),
                            (y0, x0 + 1, (1 - wy) * wx),
                            (y0 + 1, x0, wy * (1 - wx)),
                            (y0 + 1, x0 + 1, wy * wx)):
            valid = ((yi >= 0) & (yi < h_feat) & (xi >= 0) & (xi < w_feat))
            yc = np.clip(yi, 0, h_feat - 1)
            xc = np.clip(xi, 0, w_feat - 1)
            n_idx = np.arange(N)[:, None, None]
            h_idx = np.arange(Hn)[None, None, :]
            gathered = feat[n_idx, yc, xc, h_idx]
            res = res + gathered * (wgt * valid)[..., None]
        return res
    k_s = bil(k_map, sp)
    v_s = bil(v_map, sp)
    a = np.einsum('nlhd,nlhd->nlh', q, k_s) / np.sqrt(np.float32(Dn))
    a = a - a.max(axis=-1, keepdims=True)
    ex = np.exp(a)
    w = ex / ex.sum(axis=-1, keepdims=True)
    o = (w[..., None] * v_s).reshape(N, L, Cc)
    return (o @ Wout.T + bout).astype(np.float32)


def kernel(**inputs):
    query = np.asarray(inputs["query"], np.float32)
    key = np.asarray(inputs["key"], np.float32)
    value = np.asarray(inputs["value"], np.float32)
    ref_pts = np.asarray(inputs["reference_points"], np.float32)
    pos = np.asarray(inputs["pos_embed"], np.float32)
    Wq = np.asarray(inputs["Wq"], np.float32); bq = np.asarray(inputs["bq"], np.float32)
    Wk = np.asarray(inputs["Wk"], np.float32); bk = np.asarray(inputs["bk"], np.float32)
    Wv = np.asarray(inputs["Wv"], np.float32); bv = np.asarray(inputs["bv"], np.float32)
    Wp = np.asarray(inputs["Wp"], np.float32); bp = np.asarray(inputs["bp"], np.float32)
    Woff = np.asarray(inputs["Woff"], np.float32); boff = np.asarray(inputs["boff"], np.float32)
    Wout = np.asarray(inputs["Wout"], np.float32); bout = np.asarray(inputs["bout"], np.float32)
    h_feat = int(inputs["h_feat"]); w_feat = int(inputs["w_feat"])

    N, L, Cc = query.shape
    general = (np.any(Woff) or np.any(boff) or np.any(bq) or np.any(bk)
               or np.any(bv) or np.any(bp) or np.any(bout)
               or h_feat != HF or w_feat != WF or (N, L, Cc) != (N_BATCH, LMAP, C))
    if general:
        return _reference_numpy(query, key, value, ref_pts, pos,
                                Wq, bq, Wk, bk, Wv, bv, Wp, bp, Woff, boff,
                                Wout, bout, h_feat, w_feat)

    wk = np.ascontiguousarray(Wk.T).astype(NP_BF16)
    wv = np.ascontiguousarray(Wv.T).astype(NP_BF16)
    wp = np.ascontiguousarray(Wp.T).astype(NP_BF16)
    wq = np.ascontiguousarray(Wq.T).astype(NP_BF16)
    wo = np.ascontiguousarray(Wout.T).astype(NP_BF16)
    ident = np.eye(P, dtype=np.float32).astype(NP_BF16)

    # per-core host prep
    preps = []
    nt_q = 0
    for c in range(N_CORES):
        n, h = c // 2, c % 2
        tok_sorted, r0s, syx4 = host_prep(ref_pts[n], h)
        preps.append((tok_sorted, r0s, syx4))
        nt_q = max(nt_q, (tok_sorted.shape[0] + P - 1) // P)
    if nt_q % 2:
        nt_q += 1

    bounds_max = None
    core_arrays = []
    for c in range(N_CORES):
        tok_sorted, r0s, syx4 = preps[c]
        idxw, syxp, bounds = _build_core_arrays(r0s, syx4, nt_q)
        core_arrays.append((idxw, syxp))
        bounds_max = bounds if bounds_max is None else np.maximum(bounds_max, bounds)

    nc = _get_program(nt_q, bounds_max)

    def region_slice(arrT, h):
        # arrT: [C, 9216] -> [C, MAP_ROWS] region slice (zero-padded for h=1)
        if h == 0:
            return np.ascontiguousarray(arrT[:, 0:MAP_ROWS])
        out = np.zeros((C, MAP_ROWS), arrT.dtype)
        out[:, 0:LMAP - REG] = arrT[:, REG:LMAP]
        return out

    in_maps = []
    for c in range(N_CORES):
        n, h = c // 2, c % 2
        tok_sorted, _, _ = preps[c]
        idxw, syxp = core_arrays[c]
        count = tok_sorted.shape[0]
        kT = key[n].T.astype(NP_BF16)
        vT = value[n].T.astype(NP_BF16)
        pT = pos[n].T.astype(NP_BF16)
        qT = np.zeros((C, nt_q * P), NP_BF16)
        pqT = np.zeros((C, nt_q * P), NP_BF16)
        qT[:, :count] = query[n, tok_sorted].T.astype(NP_BF16)
        pqT[:, :count] = pos[n, tok_sorted].T.astype(NP_BF16)
        in_maps.append({
            "keyT": region_slice(kT, h),
            "valueT": region_slice(vT, h),
            "posT": region_slice(pT, h),
            "queryT": qT,
            "posqT": pqT,
            "idxT": idxw,
            "syxT": syxp,
            "wkT": wk, "wvT": wv, "wpT": wp, "wqT": wq, "woT": wo,
            "identity": ident,
        })

    res = run_bass_kernel_spmd(nc, in_maps, list(range(N_CORES)),
                               **_RUN_KWARGS)
    if _RESULT_HOOK is not None:
        _RESULT_HOOK(res)
    full = np.empty((N, LMAP, C), np.float32)
    for c in range(N_CORES):
        n, _ = c // 2, c % 2
        tok_sorted, _, _ = preps[c]
        count = tok_sorted.shape[0]
        o = np.asarray(res.results[c]["out"]).astype(np.float32)
        full[n, tok_sorted] = o[:count]
    return full


# test hooks (harmless defaults for standalone grading)
_RUN_KWARGS: dict = {}
_RESULT_HOOK = None


# revision 17
# speedup vs baseline: 1.1077x; 1.1077x over previous
"""Trainium2 Bass kernel v3 for nn_ExtensibleAttention.

Math (reference.py):
  q = query@Wq.T + pos@Wp.T ; k = key@Wk.T + pos@Wp.T ; v = value@Wv.T
  sp = reference_points (offsets are zero for this problem)
  k_s, v_s = bilinear_sample(k_map, sp), bilinear_sample(v_map, sp)
  a = (q.k_s)/sqrt(D) per head ; w = softmax over the 8 heads
  out = (w * v_s) @ Wout.T + bout

v3 structure (vs v2 baseline):
  - REGION SHARDING: 8 cores = 4 batches x 2 map-row regions. Core (n,h)
    builds only map rows [h*4608, h*4608+4736) (37 tiles instead of 72)
    and serves the tokens of batch n whose sample row falls in its
    region. Halves the map projection matmuls and the k/v/pos loads.
  - combine rewritten as 6 TS/TT ops (y-pair then x-pair) exploiting the
    DVE 4x TENSOR_SCALAR mode; weights passed as separate sy0/sy1/sx0/
    sx1 vectors instead of 4 products.
  - q projection just-in-time per pair; the q.k mult reads q straight
    from PSUM (no q_all copy).
  - softmax 1/sum folded into the output-projection PSUM->SBUF copy
    (ACT scale), so the weighted-v op is a plain TT with broadcast exp.
  - per-pair batching of tail ops (tmp/reduce/recip/outs) to amortize
    fixed instruction overheads; map copies batched 2 tiles per call.
  - all HWDGE dma_start on the sync queue (ACT queue freed).

Sharding: 8 cores = 4 batches x 2 map regions; tokens sorted by sampled
map row; output unsharded host-side by scatter.
"""

import sys

import numpy as np

if "/opt/trn_rl_repo" not in sys.path:
    sys.path.insert(0, "/opt/trn_rl_repo")

import concourse.bacc as bacc
import concourse.mybir as mybir
import concourse.tile as tile
from concourse import library_config
from concourse.bass_utils import run_bass_kernel_spmd
from concourse.mybir import ActivationFunctionType as AFT
from concourse.mybir import AluOpType as ALU
import bass_rust

F32 = mybir.dt.float32
BF16 = mybir.dt.bfloat16
I16 = mybir.dt.int16
NP_BF16 = mybir.dt.np(BF16)

P = 128
C = 256
CH = 2
H = 8
D = 32
HF = WF = 96
LMAP = HF * WF          # 9216
REG = LMAP // 2         # 4608 rows per region
NT_MAP = 37             # map tiles per core (region + 97-row overlap)
MAP_ROWS = NT_MAP * P   # 4736
ROW = 2 * C             # 512 bf16 elems per map row (k || v)
ELEM = 2 * ROW          # gather element: x-pair, 2 rows
N_CORES = 8
N_BATCH = 4
INV_SQRT_D = 1.0 / np.sqrt(np.float32(D))
MAP_CHUNKS = [4, 5, 9, 9, 10]   # strip 0 split so the first block flushes early


def _q_chunks(nt_q):
    base = nt_q // 4
    rem = nt_q - 4 * base
    return [base + (1 if i < rem else 0) for i in range(4)]


def build_program(nt_q, row_bounds):
    npair = nt_q // 2
    q_chunks = _q_chunks(nt_q)

    nc = bacc.Bacc("TRN2", target_bir_lowering=False, debug=False,
                   num_devices=N_CORES, num_swdge_queues=2)

    keyT = nc.dram_tensor("keyT", [C, MAP_ROWS], BF16, kind="ExternalInput")
    valueT = nc.dram_tensor("valueT", [C, MAP_ROWS], BF16, kind="ExternalInput")
    posT = nc.dram_tensor("posT", [C, MAP_ROWS], BF16, kind="ExternalInput")
    queryT = nc.dram_tensor("queryT", [C, nt_q * P], BF16, kind="ExternalInput")
    posqT = nc.dram_tensor("posqT", [C, nt_q * P], BF16, kind="ExternalInput")
    idxT = nc.dram_tensor("idxT", [npair, P, 32], I16, kind="ExternalInput")
    syxT = nc.dram_tensor("syxT", [nt_q, P, 4], F32, kind="ExternalInput")
    wkT = nc.dram_tensor("wkT", [C, C], BF16, kind="ExternalInput")
    wvT = nc.dram_tensor("wvT", [C, C], BF16, kind="ExternalInput")
    wpT = nc.dram_tensor("wpT", [C, C], BF16, kind="ExternalInput")
    wqT = nc.dram_tensor("wqT", [C, C], BF16, kind="ExternalInput")
    woT = nc.dram_tensor("woT", [C, C], BF16, kind="ExternalInput")
    identity = nc.dram_tensor("identity", [P, P], BF16, kind="ExternalInput")
    out = nc.dram_tensor("out", [nt_q * P, C], BF16, kind="ExternalOutput")

    kv_map = nc.dram_tensor("kv_map", [MAP_ROWS, ROW], BF16, kind="Internal")

    with tile.TileContext(nc) as tc:
        with (
            tc.tile_pool(name="const", bufs=1) as const,
            tc.tile_pool(name="kstrip", bufs=2) as kstrip_p,
            tc.tile_pool(name="vstrip", bufs=2) as vstrip_p,
            tc.tile_pool(name="pstrip", bufs=2) as pstrip_p,
            tc.tile_pool(name="qstrip", bufs=3) as qstrip_p,
            tc.tile_pool(name="pqstrip", bufs=3) as pqstrip_p,
            tc.tile_pool(name="kv", bufs=3) as kv_p,
            tc.tile_pool(name="gat", bufs=6) as gat_p,
            tc.tile_pool(name="att", bufs=3) as att_p,
            tc.tile_pool(name="kvs", bufs=3) as kvs_p,
            tc.tile_pool(name="small", bufs=3) as small_p,
            tc.tile_pool(name="rin", bufs=4) as rin_p,
            tc.tile_pool(name="outs", bufs=4) as outs_p,
            tc.tile_pool(name="obuf", bufs=2) as obuf_p,
            tc.tile_pool(name="psM", bufs=2, space="PSUM") as psM,
            tc.tile_pool(name="psQ", bufs=2, space="PSUM") as psQ,
            tc.tile_pool(name="psT", bufs=1, space="PSUM") as psT,
            tc.tile_pool(name="psF", bufs=1, space="PSUM") as psF,
        ):
            # ---- constants needed by the map/q strips (loaded first) ----
            def load_w(t):
                sb = const.tile([P, CH, C], BF16, tag=f"w_{t.name}")
                nc.sync.dma_start(sb[:], t.ap().rearrange("(ch p) n -> p ch n", p=P))
                return sb
            wk_sb, wv_sb, wp_sb, wq_sb = (
                load_w(t) for t in (wkT, wvT, wpT, wqT))

            nc.gpsimd.load_library(library_config.mlp)

            # constants only needed once pairs start draining; loaded after
            # the first strips so they don't delay the first map matmuls
            late = {}

            def load_late():
                late["wo_sb"] = load_w(woT)
                ident_sb = const.tile([P, P], BF16, tag="ident")
                nc.sync.dma_start(ident_sb[:], identity.ap())
                late["ident_sb"] = ident_sb
                idx_sb = const.tile([P, npair, 32], I16, tag="idx")
                nc.sync.dma_start(idx_sb[:], idxT.ap().rearrange("j p s -> p j s"))
                late["idx_sb"] = idx_sb
                syx = const.tile([P, nt_q, 4], F32, tag="syx")
                nc.sync.dma_start(syx[:], syxT.ap().rearrange("j p c -> p j c"))
                late["syx"] = syx

            # ---- map strips: project k/v/pos into kv_map rows ----
            def map_strip_loads(s):
                ls = MAP_CHUNKS[s] * P
                t0 = sum(MAP_CHUNKS[:s])
                k_st = kstrip_p.tile([P, CH, 10 * P], BF16, name="k_st")
                v_st = vstrip_p.tile([P, CH, 10 * P], BF16, name="v_st")
                p_st = pstrip_p.tile([P, CH, 10 * P], BF16, name="p_st")
                for st, t in ((k_st, keyT), (p_st, posT), (v_st, valueT)):
                    nc.sync.dma_start(
                        st[:, :, 0:ls],
                        t.ap().rearrange("(ch p) l -> p ch l", p=P)
                        [:, :, t0 * P:t0 * P + ls])
                return k_st, p_st, v_st

            def map_block(strips, s, b):
                k_st, p_st, v_st = strips
                t0 = sum(MAP_CHUNKS[:s])
                cnt = min(2, MAP_CHUNKS[s] - b)
                kv2 = psM.tile([P, 2, ROW], F32, space="PSUM", name="kv2")
                for jj in range(cnt):
                    sl = slice((b + jj) * P, (b + jj + 1) * P)
                    nc.tensor.matmul(kv2[:, jj, 0:C], k_st[:, 0, sl],
                                     wk_sb[:, 0, :], start=True, stop=False)
                    nc.tensor.matmul(kv2[:, jj, 0:C], k_st[:, 1, sl],
                                     wk_sb[:, 1, :], start=False, stop=False)
                    nc.tensor.matmul(kv2[:, jj, 0:C], p_st[:, 0, sl],
                                     wp_sb[:, 0, :], start=False, stop=False)
                    nc.tensor.matmul(kv2[:, jj, 0:C], p_st[:, 1, sl],
                                     wp_sb[:, 1, :], start=False, stop=True)
                    nc.tensor.matmul(kv2[:, jj, C:ROW], v_st[:, 0, sl],
                                     wv_sb[:, 0, :], start=True, stop=False)
                    nc.tensor.matmul(kv2[:, jj, C:ROW], v_st[:, 1, sl],
                                     wv_sb[:, 1, :], start=False, stop=True)
                kvt = kv_p.tile([P, 2, ROW], BF16, name="kvt")
                # map PSUM->SBUF copy on ACT (DVE carries the combine)
                nc.scalar.activation(kvt[:, 0:cnt, :], kv2[:, 0:cnt, :],
                                     AFT.Copy)
                m0 = t0 + b
                nc.sync.dma_start(
                    kv_map.ap()[m0 * P:(m0 + cnt) * P, :]
                    .rearrange("(jj p) e -> p jj e", p=P),
                    kvt[:, 0:cnt, :])
                return cnt

            # ---- q strips: load raw query/pos chunks (projection is JIT) ----
            q_tiles = {}

            def q_strip(s):
                ls = q_chunks[s] * P
                if ls == 0:
                    return
                t0 = sum(q_chunks[:s])
                q_st = qstrip_p.tile([P, CH, 10 * P], BF16, name="q_st")
                pq_st = pqstrip_p.tile([P, CH, 10 * P], BF16, name="pq_st")
                for st, t in ((q_st, queryT), (pq_st, posqT)):
                    nc.sync.dma_start(
                        st[:, :, 0:ls],
                        t.ap().rearrange("(ch p) l -> p ch l", p=P)
                        [:, :, t0 * P:t0 * P + ls])
                for j in range(t0, t0 + q_chunks[s]):
                    q_tiles[j] = (q_st, pq_st, (j - t0) * P)

            # ---- attention pair: gather + q proj + combine + softmax ----
            pair_state = {}

            def emit_pair(pj):
                idx_sb, syx = late["idx_sb"], late["syx"]
                j0 = 2 * pj
                g = gat_p.tile([P, 4, ELEM], BF16, name="g")
                nrows = int(row_bounds[pj])
                src = bass_rust.AP(tensor=kv_map.ap().tensor, offset=0,
                                   ap=[[ROW, nrows], [1, ELEM]])
                nc.gpsimd.dma_gather(
                    out_ap=g[:],
                    in_ap=src,
                    idxs_ap=idx_sb[:, pj, :],
                    num_idxs=4 * P,
                    num_idxs_reg=4 * P,
                    elem_size=ELEM,
                    elem_step=ROW,
                    queue_num=pj % 2,
                )
                # q projection JIT for both tiles of the pair
                qp = psQ.tile([P, 2, C], F32, space="PSUM", name="qp")
                for jj in range(2):
                    q_st, pq_st, off = q_tiles[j0 + jj]
                    sl = slice(off, off + P)
                    nc.tensor.matmul(qp[:, jj, :], q_st[:, 0, sl], wq_sb[:, 0, :],
                                     start=True, stop=False)
                    nc.tensor.matmul(qp[:, jj, :], q_st[:, 1, sl], wq_sb[:, 1, :],
                                     start=False, stop=False)
                    nc.tensor.matmul(qp[:, jj, :], pq_st[:, 0, sl], wp_sb[:, 0, :],
                                     start=False, stop=False)
                    nc.tensor.matmul(qp[:, jj, :], pq_st[:, 1, sl], wp_sb[:, 1, :],
                                     start=False, stop=True)

                kvs = kvs_p.tile([P, 2, ROW], BF16, name="kvs")
                for jj in range(2):
                    j = j0 + jj
                    g0, g1 = 2 * jj, 2 * jj + 1
                    # 4-corner combine: per-corner TS (DVE 4x mode on [512]),
                    # one corner on ACT to balance; tree of 2x TT adds.
                    # slots: g[g0]=y0 elem (x0|x1 rows), g[g1]=y1 elem.
                    # NB: gpsimd elementwise here forces Pool MODIFY_POOL_CONFIG
                    # swaps against the gather preps and serializes the whole
                    # pipeline (measured 2x slowdown) — keep off gpsimd.
                    cA = att_p.tile([P, ROW], BF16, name="cA", tag="cA")
                    nc.vector.tensor_scalar(out=cA[:], in0=g[:, g0, 0:ROW],
                                            scalar1=syx[:, j, 0:1], scalar2=None,
                                            op0=ALU.mult)
                    cB = att_p.tile([P, ROW], BF16, name="cB", tag="cB")
                    nc.scalar.activation(cB[:], g[:, g0, ROW:ELEM], AFT.Copy,
                                         scale=syx[:, j, 1:2])
                    cC = att_p.tile([P, ROW], BF16, name="cC", tag="cC")
                    nc.vector.tensor_scalar(out=cC[:], in0=g[:, g1, 0:ROW],
                                            scalar1=syx[:, j, 2:3], scalar2=None,
                                            op0=ALU.mult)
                    cD = att_p.tile([P, ROW], BF16, name="cD", tag="cD")
                    nc.scalar.activation(cD[:], g[:, g1, ROW:ELEM], AFT.Copy,
                                         scale=syx[:, j, 3:4])
                    s0 = att_p.tile([P, ROW], BF16, name="s0", tag="s0")
                    nc.vector.tensor_tensor(out=s0[:], in0=cA[:], in1=cC[:],
                                            op=ALU.add)
                    s1 = att_p.tile([P, ROW], BF16, name="s1", tag="s1")
                    nc.vector.tensor_tensor(out=s1[:], in0=cB[:], in1=cD[:],
                                            op=ALU.add)
                    nc.vector.tensor_tensor(out=kvs[:, jj, :], in0=s0[:],
                                            in1=s1[:], op=ALU.add)

                # pair-batched tail
                tmp = att_p.tile([P, 2, C], BF16, name="tmp", tag="tmp")
                nc.vector.tensor_tensor(out=tmp[:], in0=qp[:],
                                        in1=kvs[:, :, 0:C], op=ALU.mult)
                a = small_p.tile([P, 2, H], F32, name="a", tag="a")
                nc.vector.reduce_sum(
                    out=a[:], in_=tmp[:].rearrange("p t (h d) -> p t h d", d=D),
                    axis=mybir.AxisListType.X)
                e = small_p.tile([P, 2, H], F32, name="e", tag="e")
                ssum = small_p.tile([P, 2], F32, name="ssum", tag="ssum")
                for jj in range(2):
                    nc.scalar.activation(e[:, jj, :], a[:, jj, :], AFT.Exp,
                                         scale=float(INV_SQRT_D),
                                         accum_out=ssum[:, jj:jj + 1])
                rinv = rin_p.tile([P, 2], F32, name="rinv")
                nc.vector.reciprocal(rinv[:], ssum[:])
                outs = outs_p.tile([P, 2, C], BF16, name="outs")
                nc.vector.tensor_tensor(
                    out=outs[:].rearrange("p t (h d) -> p t h d", d=D),
                    in0=kvs[:, :, C:ROW].rearrange("p t (h d) -> p t h d", d=D),
                    in1=e[:].to_broadcast([P, 2, H, D]),
                    op=ALU.mult)
                pair_state[pj] = (outs, rinv)

            # ---- output projection per pair (deferred by a small lag) ----
            def emit_att_b(pj):
                ident_sb, wo_sb = late["ident_sb"], late["wo_sb"]
                outs, rinv = pair_state.pop(pj)
                tp2 = psT.tile([P, 2, C], BF16, space="PSUM", name="tp2")
                for jj in range(2):
                    nc.tensor.transpose(tp2[:, jj, 0:P], outs[:, jj, 0:P],
                                        ident_sb[:])
                    nc.tensor.transpose(tp2[:, jj, P:C], outs[:, jj, P:C],
                                        ident_sb[:])
                oT2 = att_p.tile([P, 2, C], BF16, name="oT2", tag="oT2")
                nc.scalar.activation(oT2[:], tp2[:], AFT.Copy)
                fp2 = psF.tile([P, 2, C], F32, space="PSUM", name="fp2")
                for jj in range(2):
                    nc.tensor.matmul(fp2[:, jj, :], oT2[:, jj, 0:P],
                                     wo_sb[:, 0, :], start=True, stop=False)
                    nc.tensor.matmul(fp2[:, jj, :], oT2[:, jj, P:C],
                                     wo_sb[:, 1, :], start=False, stop=True)
                ot2 = obuf_p.tile([P, 2, C], BF16, name="ot2")
                for jj in range(2):
                    nc.scalar.activation(ot2[:, jj, :], fp2[:, jj, :], AFT.Copy,
                                         scale=rinv[:, jj:jj + 1])
                j0 = 2 * pj
                nc.sync.dma_start(
                    out.ap()[j0 * P:(j0 + 2) * P, :]
                    .rearrange("(jj p) e -> p jj e", p=P),
                    ot2[:])

            # ---- schedule: fine-grained map-block / pair interleave ----
            pj = 0
            bj = 0
            flushed = 0

            def drain(limit_rows, q_loaded):
                nonlocal pj, bj
                while (pj < npair and int(row_bounds[pj]) <= limit_rows
                       and 2 * (pj + 1) * P <= q_loaded):
                    emit_pair(pj)
                    pj += 1
                    if pj - 2 > bj:
                        emit_att_b(bj)
                        bj += 1

            q_loaded = 0
            for s in range(len(MAP_CHUNKS)):
                strips = map_strip_loads(s)
                if s < 4:
                    q_strip(s)
                    q_loaded = P * sum(q_chunks[:s + 1])
                if s == 0:
                    load_late()
                for b in range(0, MAP_CHUNKS[s], 2):
                    prev = flushed
                    flushed += P * map_block(strips, s, b)
                    # one-block lag so ACT-queue combine ops don't head-block
                    # behind a still-in-flight gather
                    drain(prev, q_loaded)
            drain(MAP_ROWS, nt_q * P)
            while bj < npair:
                emit_att_b(bj)
                bj += 1

    nc.compile()
    return nc


_PROGRAM = None
_PROGRAM_KEY = None


def _get_program(nt_q, row_bounds):
    global _PROGRAM, _PROGRAM_KEY
    key = (nt_q, tuple(int(b) for b in row_bounds))
    if _PROGRAM is None or _PROGRAM_KEY != key:
        _PROGRAM_KEY = key
        _PROGRAM = build_program(nt_q, row_bounds)
    return _PROGRAM


def host_prep(ref_pts, h):
    """Region prep for one core: tokens of the batch whose bilinear base row
    falls in region h. ref_pts: [9216, 2]. Returns (tok_sorted, count,
    idx_vals [npair-var, 512] builder inputs, syx [count,4], bounds...)"""
    x = ref_pts[:, 0] * np.float32(WF) - np.float32(0.5)
    y = ref_pts[:, 1] * np.float32(HF) - np.float32(0.5)
    x0 = np.floor(x)
    y0 = np.floor(y)
    wx = (x - x0).astype(np.float32)
    wy = (y - y0).astype(np.float32)
    xb = np.clip(x0, 0, WF - 1).astype(np.int32)
    yb = np.clip(y0, 0, HF - 1).astype(np.int32)
    sy0 = np.where(y0 < 0, wy, 1.0 - wy).astype(np.float32)
    sy1 = np.where((y0 < 0) | (y0 >= HF - 1), 0.0, wy).astype(np.float32)
    sx0 = np.where(x0 < 0, wx, 1.0 - wx).astype(np.float32)
    sx1 = np.where((x0 < 0) | (x0 >= WF - 1), 0.0, wx).astype(np.float32)
    r0 = (yb * WF + xb).astype(np.int32)

    sel = np.where((r0 >= h * REG) & (r0 < (h + 1) * REG))[0]
    r0l = r0[sel] - h * REG
    order = np.argsort(r0l, kind="stable")
    tok_sorted = sel[order]
    r0s = r0l[order]
    # corner weights in gather-slot order [y0x0, y0x1, y1x0, y1x1]
    syx4 = np.stack([sy0[tok_sorted] * sx0[tok_sorted],
                     sy0[tok_sorted] * sx1[tok_sorted],
                     sy1[tok_sorted] * sx0[tok_sorted],
                     sy1[tok_sorted] * sx1[tok_sorted]], axis=1)
    return tok_sorted, r0s, syx4


def _build_core_arrays(r0s, syx4, nt_q):
    npair = nt_q // 2
    ntok = nt_q * P
    count = r0s.shape[0]
    r0p = np.zeros(ntok, np.int32)
    r0p[:count] = r0s
    syxp = np.zeros((ntok, 4), np.float32)
    syxp[:count] = syx4
    idx_wrapped = np.zeros((npair, P, 32), np.int16)
    row_bounds = np.zeros((npair,), np.int32)
    for pj in range(npair):
        vals = []
        for half in range(2):
            rt = r0p[(2 * pj + half) * P:(2 * pj + half + 1) * P]
            vals.append(rt)
            vals.append(rt + WF)
        vals = np.concatenate(vals).astype(np.int16)
        buf = np.zeros((P, 32), np.int16)
        ii = np.arange(4 * P)
        buf[ii % 16, ii // 16] = vals
        for k in range(1, 8):
            buf[16 * k:16 * (k + 1)] = buf[0:16]
        idx_wrapped[pj] = buf
        row_bounds[pj] = min(int(vals.max()) + 2, MAP_ROWS - 1)
    return idx_wrapped, syxp.reshape(nt_q, P, 4), row_bounds


def _reference_numpy(query, key, value, reference_points, pos_embed,
                     Wq, bq, Wk, bk, Wv, bv, Wp, bp, Woff, boff, Wout, bout,
                     h_feat, w_feat):
    """Exact numpy fallback (only used for non-matching setups)."""
    N, L, Cc = query.shape
    Hn = H
    Dn = Cc // Hn
    q = (query @ Wq.T + bq).reshape(N, L, Hn, Dn)
    k = (key @ Wk.T + bk).reshape(N, L, Hn, Dn)
    v = (value @ Wv.T + bv).reshape(N, L, Hn, Dn)
    pos = (pos_embed @ Wp.T + bp).reshape(N, L, Hn, Dn)
    q = q + pos
    k = k + pos
    offsets = (query @ Woff.T + boff).reshape(N, L, Hn, 2)
    sp = reference_points[:, :, None, :] + offsets
    k_map = k.reshape(N, h_feat, w_feat, Hn, Dn)
    v_map = v.reshape(N, h_feat, w_feat, Hn, Dn)

    def bil(feat, pts):
        x = pts[..., 0] * w_feat - 0.5
        y = pts[..., 1] * h_feat - 0.5
        x0 = np.floor(x).astype(np.int64)
        y0 = np.floor(y).astype(np.int64)
        wx = x - x0
        wy = y - y0
        res = 0.0
        for yi, xi, wgt in ((y0, x0, (1 - wy) * (1 - wx)),
                            (y0, x0 + 1, (1 - wy) * wx),
                            (y0 + 1, x0, wy * (1 - wx)),
                            (y0 + 1, x0 + 1, wy * wx)):
            valid = ((yi >= 0) & (yi < h_feat) & (xi >= 0) & (xi < w_feat))
            yc = np.clip(yi, 0, h_feat - 1)
            xc = np.clip(xi, 0, w_feat - 1)
            n_idx = np.arange(N)[:, None, None]
            h_idx = np.arange(Hn)[None, None, :]
            gathered = feat[n_idx, yc, xc, h_idx]
            res = res + gathered * (wgt * valid)[..., None]
        return res
    k_s = bil(k_map, sp)
    v_s = bil(v_map, sp)
    a = np.einsum('nlhd,nlhd->nlh', q, k_s) / np.sqrt(np.float32(Dn))
    a = a - a.max(axis=-1, keepdims=True)
    ex = np.exp(a)
    w = ex / ex.sum(axis=-1, keepdims=True)
    o = (w[..., None] * v_s).reshape(N, L, Cc)
    return (o @ Wout.T + bout).astype(np.float32)


def kernel(**inputs):
    query = np.asarray(inputs["query"], np.float32)
    key = np.asarray(inputs["key"], np.float32)
    value = np.asarray(inputs["value"], np.float32)
    ref_pts = np.asarray(inputs["reference_points"], np.float32)
    pos = np.asarray(inputs["pos_embed"], np.float32)
    Wq = np.asarray(inputs["Wq"], np.float32); bq = np.asarray(inputs["bq"], np.float32)
    Wk = np.asarray(inputs["Wk"], np.float32); bk = np.asarray(inputs["bk"], np.float32)
    Wv = np.asarray(inputs["Wv"], np.float32); bv = np.asarray(inputs["bv"], np.float32)
    Wp = np.asarray(inputs["Wp"], np.float32); bp = np.asarray(inputs["bp"], np.float32)
    Woff = np.asarray(inputs["Woff"], np.float32); boff = np.asarray(inputs["boff"], np.float32)
    Wout = np.asarray(inputs["Wout"], np.float32); bout = np.asarray(inputs["bout"], np.float32)
    h_feat = int(inputs["h_feat"]); w_feat = int(inputs["w_feat"])

    N, L, Cc = query.shape
    general = (np.any(Woff) or np.any(boff) or np.any(bq) or np.any(bk)
               or np.any(bv) or np.any(bp) or np.any(bout)
               or h_feat != HF or w_feat != WF or (N, L, Cc) != (N_BATCH, LMAP, C))
    if general:
        return _reference_numpy(query, key, value, ref_pts, pos,
                                Wq, bq, Wk, bk, Wv, bv, Wp, bp, Woff, boff,
                                Wout, bout, h_feat, w_feat)

    wk = np.ascontiguousarray(Wk.T).astype(NP_BF16)
    wv = np.ascontiguousarray(Wv.T).astype(NP_BF16)
    wp = np.ascontiguousarray(Wp.T).astype(NP_BF16)
    wq = np.ascontiguousarray(Wq.T).astype(NP_BF16)
    wo = np.ascontiguousarray(Wout.T).astype(NP_BF16)
    ident = np.eye(P, dtype=np.float32).astype(NP_BF16)

    # per-core host prep
    preps = []
    nt_q = 0
    for c in range(N_CORES):
        n, h = c // 2, c % 2
        tok_sorted, r0s, syx4 = host_prep(ref_pts[n], h)
        preps.append((tok_sorted, r0s, syx4))
        nt_q = max(nt_q, (tok_sorted.shape[0] + P - 1) // P)
    if nt_q % 2:
        nt_q += 1

    bounds_max = None
    core_arrays = []
    for c in range(N_CORES):
        tok_sorted, r0s, syx4 = preps[c]
        idxw, syxp, bounds = _build_core_arrays(r0s, syx4, nt_q)
        core_arrays.append((idxw, syxp))
        bounds_max = bounds if bounds_max is None else np.maximum(bounds_max, bounds)

    nc = _get_program(nt_q, bounds_max)

    def region_slice(arrT, h):
        # arrT: [C, 9216] -> [C, MAP_ROWS] region slice (zero-padded for h=1)
        if h == 0:
            return np.ascontiguousarray(arrT[:, 0:MAP_ROWS])
        out = np.zeros((C, MAP_ROWS), arrT.dtype)
        out[:, 0:LMAP - REG] = arrT[:, REG:LMAP]
        return out

    in_maps = []
    for c in range(N_CORES):
        n, h = c // 2, c % 2
        tok_sorted, _, _ = preps[c]
        idxw, syxp = core_arrays[c]
        count = tok_sorted.shape[0]
        kT = key[n].T.astype(NP_BF16)
        vT = value[n].T.astype(NP_BF16)
        pT = pos[n].T.astype(NP_BF16)
        qT = np.zeros((C, nt_q * P), NP_BF16)
        pqT = np.zeros((C, nt_q * P), NP_BF16)
        qT[:, :count] = query[n, tok_sorted].T.astype(NP_BF16)
        pqT[:, :count] = pos[n, tok_sorted].T.astype(NP_BF16)
        in_maps.append({
            "keyT": region_slice(kT, h),
            "valueT": region_slice(vT, h),
            "posT": region_slice(pT, h),
            "queryT": qT,
            "posqT": pqT,
            "idxT": idxw,
            "syxT": syxp,
            "wkT": wk, "wvT": wv, "wpT": wp, "wqT": wq, "woT": wo,
            "identity": ident,
        })

    res = run_bass_kernel_spmd(nc, in_maps, list(range(N_CORES)),
                               **_RUN_KWARGS)
    if _RESULT_HOOK is not None:
        _RESULT_HOOK(res)
    full = np.empty((N, LMAP, C), np.float32)
    for c in range(N_CORES):
        n, _ = c // 2, c % 2
        tok_sorted, _, _ = preps[c]
        count = tok_sorted.shape[0]
        o = np.asarray(res.results[c]["out"]).astype(np.float32)
        full[n, tok_sorted] = o[:count]
    return full


# test hooks (harmless defaults for standalone grading)
_RUN_KWARGS: dict = {}
_RESULT_HOOK = None


# revision 18
# speedup vs baseline: 1.2164x; 1.0982x over previous
"""Trainium2 Bass kernel v3 for nn_ExtensibleAttention.

Math (reference.py):
  q = query@Wq.T + pos@Wp.T ; k = key@Wk.T + pos@Wp.T ; v = value@Wv.T
  sp = reference_points (offsets are zero for this problem)
  k_s, v_s = bilinear_sample(k_map, sp), bilinear_sample(v_map, sp)
  a = (q.k_s)/sqrt(D) per head ; w = softmax over the 8 heads
  out = (w * v_s) @ Wout.T + bout

v3 structure (vs v2 baseline):
  - REGION SHARDING: 8 cores = 4 batches x 2 map-row regions. Core (n,h)
    builds only map rows [h*4608, h*4608+4736) (37 tiles instead of 72)
    and serves the tokens of batch n whose sample row falls in its
    region. Halves the map projection matmuls and the k/v/pos loads.
  - combine rewritten as 6 TS/TT ops (y-pair then x-pair) exploiting the
    DVE 4x TENSOR_SCALAR mode; weights passed as separate sy0/sy1/sx0/
    sx1 vectors instead of 4 products.
  - q projection just-in-time per pair; the q.k mult reads q straight
    from PSUM (no q_all copy).
  - softmax 1/sum folded into the output-projection PSUM->SBUF copy
    (ACT scale), so the weighted-v op is a plain TT with broadcast exp.
  - per-pair batching of tail ops (tmp/reduce/recip/outs) to amortize
    fixed instruction overheads; map copies batched 2 tiles per call.
  - all HWDGE dma_start on the sync queue (ACT queue freed).

Sharding: 8 cores = 4 batches x 2 map regions; tokens sorted by sampled
map row; output unsharded host-side by scatter.
"""

import sys

import numpy as np

if "/opt/trn_rl_repo" not in sys.path:
    sys.path.insert(0, "/opt/trn_rl_repo")

import concourse.bacc as bacc
import concourse.mybir as mybir
import concourse.tile as tile
from concourse import library_config
from concourse.bass_utils import run_bass_kernel_spmd
from concourse.mybir import ActivationFunctionType as AFT
from concourse.mybir import AluOpType as ALU
import bass_rust

F32 = mybir.dt.float32
BF16 = mybir.dt.bfloat16
I16 = mybir.dt.int16
NP_BF16 = mybir.dt.np(BF16)

P = 128
C = 256
CH = 2
H = 8
D = 32
HF = WF = 96
LMAP = HF * WF          # 9216
REG = LMAP // 2         # 4608 rows per region
NT_MAP = 37             # map tiles per core (region + 97-row overlap)
MAP_ROWS = NT_MAP * P   # 4736
ROW = 2 * C             # 512 bf16 elems per map row (k || v)
ELEM = 2 * ROW          # gather element: x-pair, 2 rows
N_CORES = 8
N_BATCH = 4
INV_SQRT_D = 1.0 / np.sqrt(np.float32(D))
MAP_CHUNKS = [9, 9, 9, 10]


def _q_chunks(nt_q):
    base = nt_q // 4
    rem = nt_q - 4 * base
    return [base + (1 if i < rem else 0) for i in range(4)]


def build_program(nt_q, row_bounds):
    npair = nt_q // 2
    q_chunks = _q_chunks(nt_q)

    nc = bacc.Bacc("TRN2", target_bir_lowering=False, debug=False,
                   num_devices=N_CORES, num_swdge_queues=2)

    keyT = nc.dram_tensor("keyT", [C, MAP_ROWS], BF16, kind="ExternalInput")
    valueT = nc.dram_tensor("valueT", [C, MAP_ROWS], BF16, kind="ExternalInput")
    posT = nc.dram_tensor("posT", [C, MAP_ROWS], BF16, kind="ExternalInput")
    queryT = nc.dram_tensor("queryT", [C, nt_q * P], BF16, kind="ExternalInput")
    posqT = nc.dram_tensor("posqT", [C, nt_q * P], BF16, kind="ExternalInput")
    idxT = nc.dram_tensor("idxT", [npair, P, 32], I16, kind="ExternalInput")
    syxT = nc.dram_tensor("syxT", [nt_q, P, 4], F32, kind="ExternalInput")
    wkT = nc.dram_tensor("wkT", [C, C], BF16, kind="ExternalInput")
    wvT = nc.dram_tensor("wvT", [C, C], BF16, kind="ExternalInput")
    wpT = nc.dram_tensor("wpT", [C, C], BF16, kind="ExternalInput")
    wqT = nc.dram_tensor("wqT", [C, C], BF16, kind="ExternalInput")
    woT = nc.dram_tensor("woT", [C, C], BF16, kind="ExternalInput")
    identity = nc.dram_tensor("identity", [P, P], BF16, kind="ExternalInput")
    out = nc.dram_tensor("out", [nt_q * P, C], BF16, kind="ExternalOutput")

    kv_map = nc.dram_tensor("kv_map", [MAP_ROWS, ROW], BF16, kind="Internal")

    with tile.TileContext(nc) as tc:
        with (
            tc.tile_pool(name="const", bufs=1) as const,
            tc.tile_pool(name="kstrip", bufs=2) as kstrip_p,
            tc.tile_pool(name="vstrip", bufs=2) as vstrip_p,
            tc.tile_pool(name="pstrip", bufs=2) as pstrip_p,
            tc.tile_pool(name="qstrip", bufs=3) as qstrip_p,
            tc.tile_pool(name="pqstrip", bufs=3) as pqstrip_p,
            tc.tile_pool(name="kv", bufs=3) as kv_p,
            tc.tile_pool(name="gat", bufs=6) as gat_p,
            tc.tile_pool(name="att", bufs=3) as att_p,
            tc.tile_pool(name="kvs", bufs=3) as kvs_p,
            tc.tile_pool(name="small", bufs=3) as small_p,
            tc.tile_pool(name="rin", bufs=4) as rin_p,
            tc.tile_pool(name="outs", bufs=4) as outs_p,
            tc.tile_pool(name="obuf", bufs=2) as obuf_p,
            tc.tile_pool(name="psM", bufs=2, space="PSUM") as psM,
            tc.tile_pool(name="psQ", bufs=2, space="PSUM") as psQ,
            tc.tile_pool(name="psT", bufs=1, space="PSUM") as psT,
            tc.tile_pool(name="psF", bufs=1, space="PSUM") as psF,
        ):
            # ---- constants needed by the map/q strips (loaded first) ----
            def load_w(t):
                sb = const.tile([P, CH, C], BF16, tag=f"w_{t.name}")
                nc.sync.dma_start(sb[:], t.ap().rearrange("(ch p) n -> p ch n", p=P))
                return sb
            wk_sb, wv_sb, wp_sb, wq_sb = (
                load_w(t) for t in (wkT, wvT, wpT, wqT))

            nc.gpsimd.load_library(library_config.mlp)

            # constants only needed once pairs start draining; loaded after
            # the first strips so they don't delay the first map matmuls
            late = {}

            def load_late():
                late["wo_sb"] = load_w(woT)
                ident_sb = const.tile([P, P], BF16, tag="ident")
                nc.sync.dma_start(ident_sb[:], identity.ap())
                late["ident_sb"] = ident_sb
                idx_sb = const.tile([P, npair, 32], I16, tag="idx")
                nc.sync.dma_start(idx_sb[:], idxT.ap().rearrange("j p s -> p j s"))
                late["idx_sb"] = idx_sb
                syx = const.tile([P, nt_q, 4], F32, tag="syx")
                nc.sync.dma_start(syx[:], syxT.ap().rearrange("j p c -> p j c"))
                late["syx"] = syx

            # ---- map strips: project k/v/pos into kv_map rows ----
            def map_strip_loads(s):
                ls = MAP_CHUNKS[s] * P
                t0 = sum(MAP_CHUNKS[:s])
                k_st = kstrip_p.tile([P, CH, 10 * P], BF16, name="k_st")
                v_st = vstrip_p.tile([P, CH, 10 * P], BF16, name="v_st")
                p_st = pstrip_p.tile([P, CH, 10 * P], BF16, name="p_st")
                for st, t in ((k_st, keyT), (p_st, posT), (v_st, valueT)):
                    nc.sync.dma_start(
                        st[:, :, 0:ls],
                        t.ap().rearrange("(ch p) l -> p ch l", p=P)
                        [:, :, t0 * P:t0 * P + ls])
                return k_st, p_st, v_st

            def map_block(strips, s, b):
                k_st, p_st, v_st = strips
                t0 = sum(MAP_CHUNKS[:s])
                cnt = min(2, MAP_CHUNKS[s] - b)
                kv2 = psM.tile([P, 2, ROW], F32, space="PSUM", name="kv2")
                for jj in range(cnt):
                    sl = slice((b + jj) * P, (b + jj + 1) * P)
                    nc.tensor.matmul(kv2[:, jj, 0:C], k_st[:, 0, sl],
                                     wk_sb[:, 0, :], start=True, stop=False)
                    nc.tensor.matmul(kv2[:, jj, 0:C], k_st[:, 1, sl],
                                     wk_sb[:, 1, :], start=False, stop=False)
                    nc.tensor.matmul(kv2[:, jj, 0:C], p_st[:, 0, sl],
                                     wp_sb[:, 0, :], start=False, stop=False)
                    nc.tensor.matmul(kv2[:, jj, 0:C], p_st[:, 1, sl],
                                     wp_sb[:, 1, :], start=False, stop=True)
                    nc.tensor.matmul(kv2[:, jj, C:ROW], v_st[:, 0, sl],
                                     wv_sb[:, 0, :], start=True, stop=False)
                    nc.tensor.matmul(kv2[:, jj, C:ROW], v_st[:, 1, sl],
                                     wv_sb[:, 1, :], start=False, stop=True)
                kvt = kv_p.tile([P, 2, ROW], BF16, name="kvt")
                # map PSUM->SBUF copy on ACT (DVE carries the combine)
                nc.scalar.activation(kvt[:, 0:cnt, :], kv2[:, 0:cnt, :],
                                     AFT.Copy)
                m0 = t0 + b
                nc.sync.dma_start(
                    kv_map.ap()[m0 * P:(m0 + cnt) * P, :]
                    .rearrange("(jj p) e -> p jj e", p=P),
                    kvt[:, 0:cnt, :])
                return cnt

            # ---- q strips: load raw query/pos chunks (projection is JIT) ----
            q_tiles = {}

            def q_strip(s):
                ls = q_chunks[s] * P
                if ls == 0:
                    return
                t0 = sum(q_chunks[:s])
                q_st = qstrip_p.tile([P, CH, 10 * P], BF16, name="q_st")
                pq_st = pqstrip_p.tile([P, CH, 10 * P], BF16, name="pq_st")
                for st, t in ((q_st, queryT), (pq_st, posqT)):
                    nc.sync.dma_start(
                        st[:, :, 0:ls],
                        t.ap().rearrange("(ch p) l -> p ch l", p=P)
                        [:, :, t0 * P:t0 * P + ls])
                for j in range(t0, t0 + q_chunks[s]):
                    q_tiles[j] = (q_st, pq_st, (j - t0) * P)

            # ---- attention pair: gather + q proj + combine + softmax ----
            pair_state = {}

            def emit_pair(pj):
                idx_sb, syx = late["idx_sb"], late["syx"]
                j0 = 2 * pj
                g = gat_p.tile([P, 4, ELEM], BF16, name="g")
                nrows = int(row_bounds[pj])
                src = bass_rust.AP(tensor=kv_map.ap().tensor, offset=0,
                                   ap=[[ROW, nrows], [1, ELEM]])
                nc.gpsimd.dma_gather(
                    out_ap=g[:],
                    in_ap=src,
                    idxs_ap=idx_sb[:, pj, :],
                    num_idxs=4 * P,
                    num_idxs_reg=4 * P,
                    elem_size=ELEM,
                    elem_step=ROW,
                    queue_num=pj % 2,
                )
                # q projection JIT for both tiles of the pair
                qp = psQ.tile([P, 2, C], F32, space="PSUM", name="qp")
                for jj in range(2):
                    q_st, pq_st, off = q_tiles[j0 + jj]
                    sl = slice(off, off + P)
                    nc.tensor.matmul(qp[:, jj, :], q_st[:, 0, sl], wq_sb[:, 0, :],
                                     start=True, stop=False)
                    nc.tensor.matmul(qp[:, jj, :], q_st[:, 1, sl], wq_sb[:, 1, :],
                                     start=False, stop=False)
                    nc.tensor.matmul(qp[:, jj, :], pq_st[:, 0, sl], wp_sb[:, 0, :],
                                     start=False, stop=False)
                    nc.tensor.matmul(qp[:, jj, :], pq_st[:, 1, sl], wp_sb[:, 1, :],
                                     start=False, stop=True)

                kvs = kvs_p.tile([P, 2, ROW], BF16, name="kvs")
                for jj in range(2):
                    j = j0 + jj
                    g0, g1 = 2 * jj, 2 * jj + 1
                    # 4-corner combine: per-corner TS (DVE 4x mode on [512]),
                    # one corner on ACT to balance; tree of 2x TT adds.
                    # slots: g[g0]=y0 elem (x0|x1 rows), g[g1]=y1 elem.
                    # NB: gpsimd elementwise here forces Pool MODIFY_POOL_CONFIG
                    # swaps against the gather preps and serializes the whole
                    # pipeline (measured 2x slowdown) — keep off gpsimd.
                    cA = att_p.tile([P, ROW], BF16, name="cA", tag="cA")
                    nc.vector.tensor_scalar(out=cA[:], in0=g[:, g0, 0:ROW],
                                            scalar1=syx[:, j, 0:1], scalar2=None,
                                            op0=ALU.mult)
                    cB = att_p.tile([P, ROW], BF16, name="cB", tag="cB")
                    nc.scalar.activation(cB[:], g[:, g0, ROW:ELEM], AFT.Copy,
                                         scale=syx[:, j, 1:2])
                    cC = att_p.tile([P, ROW], BF16, name="cC", tag="cC")
                    nc.vector.tensor_scalar(out=cC[:], in0=g[:, g1, 0:ROW],
                                            scalar1=syx[:, j, 2:3], scalar2=None,
                                            op0=ALU.mult)
                    cD = att_p.tile([P, ROW], BF16, name="cD", tag="cD")
                    nc.vector.tensor_scalar(out=cD[:], in0=g[:, g1, ROW:ELEM],
                                            scalar1=syx[:, j, 3:4], scalar2=None,
                                            op0=ALU.mult)
                    s0 = att_p.tile([P, ROW], BF16, name="s0", tag="s0")
                    nc.vector.tensor_tensor(out=s0[:], in0=cA[:], in1=cC[:],
                                            op=ALU.add)
                    s1 = att_p.tile([P, ROW], BF16, name="s1", tag="s1")
                    nc.vector.tensor_tensor(out=s1[:], in0=cB[:], in1=cD[:],
                                            op=ALU.add)
                    nc.vector.tensor_tensor(out=kvs[:, jj, :], in0=s0[:],
                                            in1=s1[:], op=ALU.add)

                # pair-batched tail
                tmp = att_p.tile([P, 2, C], BF16, name="tmp", tag="tmp")
                nc.vector.tensor_tensor(out=tmp[:], in0=qp[:],
                                        in1=kvs[:, :, 0:C], op=ALU.mult)
                a = small_p.tile([P, 2, H], F32, name="a", tag="a")
                nc.vector.reduce_sum(
                    out=a[:], in_=tmp[:].rearrange("p t (h d) -> p t h d", d=D),
                    axis=mybir.AxisListType.X)
                e = small_p.tile([P, 2, H], F32, name="e", tag="e")
                ssum = small_p.tile([P, 2], F32, name="ssum", tag="ssum")
                for jj in range(2):
                    nc.scalar.activation(e[:, jj, :], a[:, jj, :], AFT.Exp,
                                         scale=float(INV_SQRT_D),
                                         accum_out=ssum[:, jj:jj + 1])
                rinv = rin_p.tile([P, 2], F32, name="rinv")
                nc.vector.reciprocal(rinv[:], ssum[:])
                outs = outs_p.tile([P, 2, C], BF16, name="outs")
                nc.vector.tensor_tensor(
                    out=outs[:].rearrange("p t (h d) -> p t h d", d=D),
                    in0=kvs[:, :, C:ROW].rearrange("p t (h d) -> p t h d", d=D),
                    in1=e[:].to_broadcast([P, 2, H, D]),
                    op=ALU.mult)
                pair_state[pj] = (outs, rinv)

            # ---- output projection per pair (deferred by a small lag) ----
            def emit_att_b(pj):
                ident_sb, wo_sb = late["ident_sb"], late["wo_sb"]
                outs, rinv = pair_state.pop(pj)
                tp2 = psT.tile([P, 2, C], BF16, space="PSUM", name="tp2")
                for jj in range(2):
                    nc.tensor.transpose(tp2[:, jj, 0:P], outs[:, jj, 0:P],
                                        ident_sb[:])
                    nc.tensor.transpose(tp2[:, jj, P:C], outs[:, jj, P:C],
                                        ident_sb[:])
                oT2 = att_p.tile([P, 2, C], BF16, name="oT2", tag="oT2")
                nc.scalar.activation(oT2[:], tp2[:], AFT.Copy)
                fp2 = psF.tile([P, 2, C], F32, space="PSUM", name="fp2")
                for jj in range(2):
                    nc.tensor.matmul(fp2[:, jj, :], oT2[:, jj, 0:P],
                                     wo_sb[:, 0, :], start=True, stop=False)
                    nc.tensor.matmul(fp2[:, jj, :], oT2[:, jj, P:C],
                                     wo_sb[:, 1, :], start=False, stop=True)
                ot2 = obuf_p.tile([P, 2, C], BF16, name="ot2")
                for jj in range(2):
                    nc.scalar.activation(ot2[:, jj, :], fp2[:, jj, :], AFT.Copy,
                                         scale=rinv[:, jj:jj + 1])
                j0 = 2 * pj
                nc.sync.dma_start(
                    out.ap()[j0 * P:(j0 + 2) * P, :]
                    .rearrange("(jj p) e -> p jj e", p=P),
                    ot2[:])

            # ---- schedule: fine-grained map-block / pair interleave ----
            pj = 0
            bj = 0
            flushed = 0

            def drain(limit_rows, q_loaded):
                nonlocal pj, bj
                while (pj < npair and int(row_bounds[pj]) <= limit_rows
                       and 2 * (pj + 1) * P <= q_loaded):
                    emit_pair(pj)
                    pj += 1
                    if pj - 2 > bj:
                        emit_att_b(bj)
                        bj += 1

            q_loaded = 0
            for s in range(len(MAP_CHUNKS)):
                strips = map_strip_loads(s)
                if s < 4:
                    q_strip(s)
                    q_loaded = P * sum(q_chunks[:s + 1])
                if s == 0:
                    load_late()
                for b in range(0, MAP_CHUNKS[s], 2):
                    prev = flushed
                    flushed += P * map_block(strips, s, b)
                    # one-block lag so ACT-queue combine ops don't head-block
                    # behind a still-in-flight gather
                    drain(prev, q_loaded)
            drain(MAP_ROWS, nt_q * P)
            while bj < npair:
                emit_att_b(bj)
                bj += 1

    nc.compile()
    return nc


_PROGRAM = None
_PROGRAM_KEY = None


def _get_program(nt_q, row_bounds):
    global _PROGRAM, _PROGRAM_KEY
    key = (nt_q, tuple(int(b) for b in row_bounds))
    if _PROGRAM is None or _PROGRAM_KEY != key:
        _PROGRAM_KEY = key
        _PROGRAM = build_program(nt_q, row_bounds)
    return _PROGRAM


def host_prep(ref_pts, h):
    """Region prep for one core: tokens of the batch whose bilinear base row
    falls in region h. ref_pts: [9216, 2]. Returns (tok_sorted, count,
    idx_vals [npair-var, 512] builder inputs, syx [count,4], bounds...)"""
    x = ref_pts[:, 0] * np.float32(WF) - np.float32(0.5)
    y = ref_pts[:, 1] * np.float32(HF) - np.float32(0.5)
    x0 = np.floor(x)
    y0 = np.floor(y)
    wx = (x - x0).astype(np.float32)
    wy = (y - y0).astype(np.float32)
    xb = np.clip(x0, 0, WF - 1).astype(np.int32)
    yb = np.clip(y0, 0, HF - 1).astype(np.int32)
    sy0 = np.where(y0 < 0, wy, 1.0 - wy).astype(np.float32)
    sy1 = np.where((y0 < 0) | (y0 >= HF - 1), 0.0, wy).astype(np.float32)
    sx0 = np.where(x0 < 0, wx, 1.0 - wx).astype(np.float32)
    sx1 = np.where((x0 < 0) | (x0 >= WF - 1), 0.0, wx).astype(np.float32)
    r0 = (yb * WF + xb).astype(np.int32)

    sel = np.where((r0 >= h * REG) & (r0 < (h + 1) * REG))[0]
    r0l = r0[sel] - h * REG
    order = np.argsort(r0l, kind="stable")
    tok_sorted = sel[order]
    r0s = r0l[order]
    # corner weights in gather-slot order [y0x0, y0x1, y1x0, y1x1]
    syx4 = np.stack([sy0[tok_sorted] * sx0[tok_sorted],
                     sy0[tok_sorted] * sx1[tok_sorted],
                     sy1[tok_sorted] * sx0[tok_sorted],
                     sy1[tok_sorted] * sx1[tok_sorted]], axis=1)
    return tok_sorted, r0s, syx4


def _build_core_arrays(r0s, syx4, nt_q):
    npair = nt_q // 2
    ntok = nt_q * P
    count = r0s.shape[0]
    r0p = np.zeros(ntok, np.int32)
    r0p[:count] = r0s
    syxp = np.zeros((ntok, 4), np.float32)
    syxp[:count] = syx4
    idx_wrapped = np.zeros((npair, P, 32), np.int16)
    row_bounds = np.zeros((npair,), np.int32)
    for pj in range(npair):
        vals = []
        for half in range(2):
            rt = r0p[(2 * pj + half) * P:(2 * pj + half + 1) * P]
            vals.append(rt)
            vals.append(rt + WF)
        vals = np.concatenate(vals).astype(np.int16)
        buf = np.zeros((P, 32), np.int16)
        ii = np.arange(4 * P)
        buf[ii % 16, ii // 16] = vals
        for k in range(1, 8):
            buf[16 * k:16 * (k + 1)] = buf[0:16]
        idx_wrapped[pj] = buf
        row_bounds[pj] = min(int(vals.max()) + 2, MAP_ROWS - 1)
    return idx_wrapped, syxp.reshape(nt_q, P, 4), row_bounds


def _reference_numpy(query, key, value, reference_points, pos_embed,
                     Wq, bq, Wk, bk, Wv, bv, Wp, bp, Woff, boff, Wout, bout,
                     h_feat, w_feat):
    """Exact numpy fallback (only used for non-matching setups)."""
    N, L, Cc = query.shape
    Hn = H
    Dn = Cc // Hn
    q = (query @ Wq.T + bq).reshape(N, L, Hn, Dn)
    k = (key @ Wk.T + bk).reshape(N, L, Hn, Dn)
    v = (value @ Wv.T + bv).reshape(N, L, Hn, Dn)
    pos = (pos_embed @ Wp.T + bp).reshape(N, L, Hn, Dn)
    q = q + pos
    k = k + pos
    offsets = (query @ Woff.T + boff).reshape(N, L, Hn, 2)
    sp = reference_points[:, :, None, :] + offsets
    k_map = k.reshape(N, h_feat, w_feat, Hn, Dn)
    v_map = v.reshape(N, h_feat, w_feat, Hn, Dn)

    def bil(feat, pts):
        x = pts[..., 0] * w_feat - 0.5
        y = pts[..., 1] * h_feat - 0.5
        x0 = np.floor(x).astype(np.int64)
        y0 = np.floor(y).astype(np.int64)
        wx = x - x0
        wy = y - y0
        res = 0.0
        for yi, xi, wgt in ((y0, x0, (1 - wy) * (1 - wx)),
                            (y0, x0 + 1, (1 - wy) * wx),
                            (y0 + 1, x0, wy * (1 - wx)),
                            (y0 + 1, x0 + 1, wy * wx)):
            valid = ((yi >= 0) & (yi < h_feat) & (xi >= 0) & (xi < w_feat))
            yc = np.clip(yi, 0, h_feat - 1)
            xc = np.clip(xi, 0, w_feat - 1)
            n_idx = np.arange(N)[:, None, None]
            h_idx = np.arange(Hn)[None, None, :]
            gathered = feat[n_idx, yc, xc, h_idx]
            res = res + gathered * (wgt * valid)[..., None]
        return res
    k_s = bil(k_map, sp)
    v_s = bil(v_map, sp)
    a = np.einsum('nlhd,nlhd->nlh', q, k_s) / np.sqrt(np.float32(Dn))
    a = a - a.max(axis=-1, keepdims=True)
    ex = np.exp(a)
    w = ex / ex.sum(axis=-1, keepdims=True)
    o = (w[..., None] * v_s).reshape(N, L, Cc)
    return (o @ Wout.T + bout).astype(np.float32)


def kernel(**inputs):
    query = np.asarray(inputs["query"], np.float32)
    key = np.asarray(inputs["key"], np.float32)
    value = np.asarray(inputs["value"], np.float32)
    ref_pts = np.asarray(inputs["reference_points"], np.float32)
    pos = np.asarray(inputs["pos_embed"], np.float32)
    Wq = np.asarray(inputs["Wq"], np.float32); bq = np.asarray(inputs["bq"], np.float32)
    Wk = np.asarray(inputs["Wk"], np.float32); bk = np.asarray(inputs["bk"], np.float32)
    Wv = np.asarray(inputs["Wv"], np.float32); bv = np.asarray(inputs["bv"], np.float32)
    Wp = np.asarray(inputs["Wp"], np.float32); bp = np.asarray(inputs["bp"], np.float32)
    Woff = np.asarray(inputs["Woff"], np.float32); boff = np.asarray(inputs["boff"], np.float32)
    Wout = np.asarray(inputs["Wout"], np.float32); bout = np.asarray(inputs["bout"], np.float32)
    h_feat = int(inputs["h_feat"]); w_feat = int(inputs["w_feat"])

    N, L, Cc = query.shape
    general = (np.any(Woff) or np.any(boff) or np.any(bq) or np.any(bk)
               or np.any(bv) or np.any(bp) or np.any(bout)
               or h_feat != HF or w_feat != WF or (N, L, Cc) != (N_BATCH, LMAP, C))
    if general:
        return _reference_numpy(query, key, value, ref_pts, pos,
                                Wq, bq, Wk, bk, Wv, bv, Wp, bp, Woff, boff,
                                Wout, bout, h_feat, w_feat)

    wk = np.ascontiguousarray(Wk.T).astype(NP_BF16)
    wv = np.ascontiguousarray(Wv.T).astype(NP_BF16)
    wp = np.ascontiguousarray(Wp.T).astype(NP_BF16)
    wq = np.ascontiguousarray(Wq.T).astype(NP_BF16)
    wo = np.ascontiguousarray(Wout.T).astype(NP_BF16)
    ident = np.eye(P, dtype=np.float32).astype(NP_BF16)

    # per-core host prep
    preps = []
    nt_q = 0
    for c in range(N_CORES):
        n, h = c // 2, c % 2
        tok_sorted, r0s, syx4 = host_prep(ref_pts[n], h)
        preps.append((tok_sorted, r0s, syx4))
        nt_q = max(nt_q, (tok_sorted.shape[0] + P - 1) // P)
    if nt_q % 2:
        nt_q += 1

    bounds_max = None
    core_arrays = []
    for c in range(N_CORES):
        tok_sorted, r0s, syx4 = preps[c]
        idxw, syxp, bounds = _build_core_arrays(r0s, syx4, nt_q)
        core_arrays.append((idxw, syxp))
        bounds_max = bounds if bounds_max is None else np.maximum(bounds_max, bounds)

    nc = _get_program(nt_q, bounds_max)

    def region_slice(arrT, h):
        # arrT: [C, 9216] -> [C, MAP_ROWS] region slice (zero-padded for h=1)
        if h == 0:
            return np.ascontiguousarray(arrT[:, 0:MAP_ROWS])
        out = np.zeros((C, MAP_ROWS), arrT.dtype)
        out[:, 0:LMAP - REG] = arrT[:, REG:LMAP]
        return out

    in_maps = []
    for c in range(N_CORES):
        n, h = c // 2, c % 2
        tok_sorted, _, _ = preps[c]
        idxw, syxp = core_arrays[c]
        count = tok_sorted.shape[0]
        kT = key[n].T.astype(NP_BF16)
        vT = value[n].T.astype(NP_BF16)
        pT = pos[n].T.astype(NP_BF16)
        qT = np.zeros((C, nt_q * P), NP_BF16)
        pqT = np.zeros((C, nt_q * P), NP_BF16)
        qT[:, :count] = query[n, tok_sorted].T.astype(NP_BF16)
        pqT[:, :count] = pos[n, tok_sorted].T.astype(NP_BF16)
        in_maps.append({
            "keyT": region_slice(kT, h),
            "valueT": region_slice(vT, h),
            "posT": region_slice(pT, h),
            "queryT": qT,
            "posqT": pqT,
            "idxT": idxw,
            "syxT": syxp,
            "wkT": wk, "wvT": wv, "wpT": wp, "wqT": wq, "woT": wo,
            "identity": ident,
        })

    res = run_bass_kernel_spmd(nc, in_maps, list(range(N_CORES)),
                               **_RUN_KWARGS)
    if _RESULT_HOOK is not None:
        _RESULT_HOOK(res)
    full = np.empty((N, LMAP, C), np.float32)
    for c in range(N_CORES):
        n, _ = c // 2, c % 2
        tok_sorted, _, _ = preps[c]
        count = tok_sorted.shape[0]
        o = np.asarray(res.results[c]["out"]).astype(np.float32)
        full[n, tok_sorted] = o[:count]
    return full


# test hooks (harmless defaults for standalone grading)
_RUN_KWARGS: dict = {}
_RESULT_HOOK = None
